# revision 14
# baseline (speedup 1.0000x reference)
"""DTW layer (short kernel) Trainium2 Bass kernel — wavefront version.

Problem: x (B=8, C=8, L=4096) f32, kernels (F=32, K=10) f32.
For each (b, c, f, w): DTW cost between kernels[f] (len 10) and window
x[b, c, 5w : 5w+20], for w in [0, 815). Output (B, C*F, 815) f32.

Sharding: data-parallel over batch — core b computes batch b entirely
(C*F = 256 (c,f) combos = 2 partition chunks of 128).

Algorithm (per core): anti-diagonal wavefront in NEGATED form.  With
A = -acc, the DTW recurrence
    acc[i,j] = D[i,j] + min(acc[i,j-1], acc[i-1,j], acc[i-1,j-1])
becomes
    A[i,j] = max(A[i,j-1], A[i-1,j], A[i-1,j-1]) - D[i,j].
Processing anti-diagonals d = i+j removes the serial chain: every cell
of a diagonal is computed by three plain f16 tensor_tensor ops (two
max, one subtract) over [rows x 136 windows] at once.  Packed-f16
tensor_tensor runs in the DVE 2x_1P mode (~0.42 ns/elem) vs the
tensor_tensor_scan recurrence at ~2 ns/elem.

HW facts this schedule is built around (measured):
  - ACT activation with a stride-5 input AP runs ~2.2x slower than
    with packed inputs, so windows are pre-expanded once per chunk
    pair into Xw[j][w] (f16) and the 10 per-row Square ops read Xw
    packed.
  - every wait_ge costs ~220 ns on the waiting engine even when
    already satisfied, so DVE waits once per unit (ACT runs a full
    unit ahead via the double-buffered D tile), not once per diagonal.

Layouts (per (cc, wc) unit; Wc=136 windows, 12 units):
  D[slot][i][j][w]   f16, flat (20*i + j)*Wc + w.  ACT writes one
                     activation(Square, bias=-k_i) op per row i; DVE
                     reads diagonal d with the affine stride 19*Wc
                     (addr = (19*i + d)*Wc + w).
  V[cc][s][r][w]     f16 ring of 2 diagonal buffers (s = d mod 2),
                     rows r = i+1 with a persistent -BIG pad row at
                     r=0 (reads of row i-1 = -1 hit the pad).  Depth 2
                     is safe: within step d, op_b reads the V_{d-2}
                     rows of slot d%2 before op_c overwrites them
                     (same-engine program order).
Boundary cells j=0 (up-only) get a dedicated 1-row subtract op; row 0
(left-only) falls out of the general op because up/diag hit the pad.
The d=28 subtract writes D - M2 = +acc[9,19] straight into the f32
output buffer.

Raw bass (no Tile framework): this toolchain's walrus codegen allows at
most 2 embedded sync-waits per instruction and rejects Tile's tail
drain, so engines are programmed directly with standalone wait_ge
instructions and per-engine semaphores.
"""

from contextlib import ExitStack

import numpy as np

import concourse.bass as bass
import concourse.mybir as mybir
from concourse.bass_utils import run_bass_kernel_spmd

# Problem constants (hardcoded per harness contract)
B, C, L = 8, 8, 4096
F, K = 32, 10
PROC, STEP = 20, 5
NW = 815          # windows actually computed == chan_outlen
NWC = 136         # windows per chunk; 6 chunks = 816 >= 815
NCHUNK = 6
ND = K + PROC - 1  # 29 anti-diagonals
BIG = 30000.0      # f16-safe sentinel
UNITS = [(wc % 2, wc // 2) for wc in range(2 * NCHUNK)]  # (cc, wchunk)
XWW = 2 * NWC      # windows per Xw expansion (one chunk pair)

F32 = mybir.dt.float32
F16 = mybir.dt.float16


def _irange(d):
    """General-op rows at diagonal d (j=0 cells excluded)."""
    return max(0, d - (PROC - 1)), min(K - 1, d - 1)


def _build_nc(reps: int = 1, small_a: bool = False, small_b: bool = False,
              small_c: bool = False, small_act: bool = False,
              fake_flat_d: bool = False, inplace_b: bool = False,
              dpad: int = 0) -> bass.Bass:
    """small_*: shrink one op class to 4 elements (timing attribution).
    fake_flat_d: op_c reads a flat D region (WRONG results; isolates the
    diagonal-stride read cost).  inplace_b: op_b maxes into M1 in place
    and skips row 0 (whose diag arg is the pad).  dpad: extra elements
    of padding per D row block (shifts the diagonal read stride).
    reps > 1 replicates the schedule (slope-based timing)."""
    # detect_race_conditions=False: CoreSim's detector does not model
    # same-engine program order, which this kernel relies on throughout.
    nc = bass.Bass("TRN2", debug=False, detect_race_conditions=False)
    x_d = nc.dram_tensor("x", [C, L], F32, kind="ExternalInput").ap()
    k_d = nc.dram_tensor("negk", [F, K], F32, kind="ExternalInput").ap()
    out_d = nc.dram_tensor("out", [C * F, NWC * NCHUNK], F32,
                           kind="ExternalOutput").ap()

    UNITS_R = UNITS * reps
    NU = len(UNITS_R)
    VROW = NWC
    VSLOT = (K + 1) * NWC       # 11 rows: pad at row 0
    DROW = PROC * NWC + dpad    # D row i block
    DDIAG = DROW - NWC          # diagonal read stride over i

    # ACT emission bookkeeping: builds (one per (cc, chunk-pair) visit)
    # interleave with the 10 row ops per unit; DVE waits on per-unit
    # totals.
    act_total = []  # act_total[u] = act_sem count after unit u's rows
    n = 0
    for u, (cc, wc) in enumerate(UNITS_R):
        if wc % 2 == 0:
            n += 1  # Xw build op
        n += K
        act_total.append(n)

    with ExitStack() as ctx:
        sb = lambda shape, name, dt: ctx.enter_context(
            nc.sbuf_tensor(name, shape, dt))
        X = [sb([128, L], f"Xt{cc}", F32) for cc in range(2)]
        negK = sb([128, K], "negKt", F32)
        D = [sb([128, K * DROW], f"Dt{s}", F16) for s in range(2)]
        V = [sb([128, 2 * VSLOT], f"Vt{cc}", F16) for cc in range(2)]
        Xw = [sb([128, PROC * XWW], f"Xwt{cc}", F16) for cc in range(2)]
        M1 = sb([128, K * NWC], "M1t", F16)
        M2 = sb([128, K * NWC], "M2t", F16)
        ZROW = sb([128, NWC], "Zt", F16)
        OB = [sb([128, NWC], f"OBt{s}", F32) for s in range(2)]

        dma_sem = ctx.enter_context(nc.semaphore("dma_sem"))
        dma0_sem = ctx.enter_context(nc.semaphore("dma0_sem"))
        act_sem = ctx.enter_context(nc.semaphore("act_sem"))
        dve_sem = ctx.enter_context(nc.semaphore("dve_sem"))
        block = ctx.enter_context(nc.Block())

        def vap(cc, slot, row, nrows=1, w=NWC):
            """AP into V[cc]: rows row..row+nrows-1 of ring slot (row -1
            = pad).  Rows are contiguous, so multi-row spans flatten to
            one free dim (multi-dim contiguous APs measured ~10x slower
            on DVE than the equivalent flat AP)."""
            t = V[cc].ap()
            off = t.offset + slot * VSLOT + (row + 1) * VROW
            if nrows > 1 and w == NWC:
                return bass.AP(t.tensor, off,
                               [list(t.ap[0]), [1, nrows * NWC]])
            if nrows == 1:
                return bass.AP(t.tensor, off, [list(t.ap[0]), [1, w]])
            return bass.AP(t.tensor, off,
                           [list(t.ap[0]), [VROW, nrows], [1, w]])

        def dap(su, off_elem, nrows=1, rstride=0, w=NWC):
            t = D[su].ap()
            off = t.offset + off_elem
            if nrows == 1:
                return bass.AP(t.tensor, off, [list(t.ap[0]), [1, w]])
            return bass.AP(t.tensor, off,
                           [list(t.ap[0]), [rstride, nrows], [1, w]])

        def map_(tile, row, nrows, w=NWC):
            t = tile.ap()
            off = t.offset + row * NWC
            if nrows > 1 and w == NWC:
                return bass.AP(t.tensor, off,
                               [list(t.ap[0]), [1, nrows * NWC]])
            if nrows == 1:
                return bass.AP(t.tensor, off, [list(t.ap[0]), [1, w]])
            return bass.AP(t.tensor, off,
                           [list(t.ap[0]), [NWC, nrows], [1, w]])

        @block.sync
        def _(sync):
            # negK + X0 first so cc0 compute starts before X1 lands.
            # X[cc] partition p holds x[4*cc + p//32, :] (source AP
            # replicates each channel row 32x via a step-0 dim)
            ksrc = bass.AP(k_d.tensor, 0, [[0, 4], [K, F], [1, K]])
            sync.dma_start(negK.ap(), ksrc).then_inc(dma0_sem, 16)
            for cc in range(2):
                src = bass.AP(x_d.tensor, 4 * cc * L,
                              [[L, 4], [0, 32], [1, L]])
                sync.dma_start(X[cc].ap(), src).then_inc(
                    dma0_sem if cc == 0 else dma_sem, 16)
            for u, (cc, wc) in enumerate(UNITS_R):
                su = u % 2
                sync.wait_ge(dve_sem, u + 1)
                sync.dma_start(
                    out_d[128 * cc:128 * (cc + 1),
                          NWC * wc:NWC * (wc + 1)],
                    OB[su].ap()).then_inc(dma_sem, 16)

        @block.scalar
        def _(scalar):
            scalar.wait_ge(dma0_sem, 32)  # negK + X0
            x1_waited = False
            dve_waited = 0
            for u, (cc, wc) in enumerate(UNITS_R):
                su = u % 2
                if cc == 1 and not x1_waited:
                    scalar.wait_ge(dma_sem, 16)  # X1
                    x1_waited = True
                if u >= 2:
                    # D[su] reads of unit u-2 fully drained
                    need = u - 1
                    if need > dve_waited:
                        scalar.wait_ge(dve_sem, need)
                        dve_waited = need
                if wc % 2 == 0:
                    # expand windows of chunk pair (wc, wc+1) into
                    # Xw[cc][j][w]: the one strided-input (slow) ACT op
                    xt = X[cc].ap()
                    src = bass.AP(xt.tensor,
                                  xt.offset + STEP * NWC * wc,
                                  [list(xt.ap[0]), [1, PROC],
                                   [STEP, XWW]])
                    scalar.copy(Xw[cc].ap(), src).then_inc(act_sem, 1)
                xw = Xw[cc].ap()
                for i in range(K):
                    win = bass.AP(xw.tensor,
                                  xw.offset + NWC * (wc % 2),
                                  [list(xw.ap[0]), [XWW, PROC],
                                   [1, NWC]])
                    dst = bass.AP(D[su].ap().tensor,
                                  D[su].ap().offset + i * DROW,
                                  [list(D[su].ap().ap[0]),
                                   [1, PROC * NWC]])
                    if small_act:
                        win = bass.AP(win.tensor, win.offset,
                                      [list(win.ap[0]), [XWW, 1], [1, 4]])
                        dst = bass.AP(dst.tensor, dst.offset,
                                      [list(dst.ap[0]), [NWC, 1], [1, 4]])
                    scalar.activation(
                        dst, win, mybir.ActivationFunctionType.Square,
                        bias=negK.ap()[:, i:i + 1],
                        scale=1.0).then_inc(act_sem, 1)

        @block.vector
        def _(vector):
            # init: V pad rows = -BIG (never overwritten), ZROW = 0
            for cc in range(2):
                for s in range(2):
                    vector.memset(vap(cc, s, -1), -BIG)
            vector.memset(ZROW.ap(), 0.0)
            act_waited = 0
            dma_waited = 0
            for u, (cc, wc) in enumerate(UNITS_R):
                su = u % 2
                need = act_total[u]
                if need > act_waited:
                    vector.wait_ge(act_sem, need)
                    act_waited = need
                for d in range(ND):
                    if d == 0:
                        # A[0,0] = 0 - D[0,0]
                        vector.tensor_tensor(
                            vap(cc, 0, 0), ZROW.ap(), dap(su, 0),
                            mybir.AluOpType.subtract)
                        continue
                    s0, s1 = d % 2, (d - 1) % 2
                    if d <= K - 1:
                        # j=0 cell, up-only: A[d,0] = A[d-1,0] - D[d,0]
                        vector.tensor_tensor(
                            vap(cc, s0, d), vap(cc, s1, d - 1),
                            dap(su, d * DROW),
                            mybir.AluOpType.subtract)
                    gl, gh = _irange(d)
                    n = gh - gl + 1
                    w = 4 if small_a else NWC
                    vector.tensor_tensor(
                        map_(M1, gl, n, w),
                        vap(cc, s1, gl, n, w=w),
                        vap(cc, s1, gl - 1, n, w=w),
                        mybir.AluOpType.max)
                    w = 4 if small_b else NWC
                    if inplace_b:
                        # row 0's diag arg is the pad: M1 row 0 already
                        # final; max the rest into M1 in place
                        bl = max(gl, 1)
                        if bl <= gh:
                            nb = gh - bl + 1
                            vector.tensor_tensor(
                                map_(M1, bl, nb, w),
                                map_(M1, bl, nb, w),
                                vap(cc, s0, bl - 1, nb, w=w),
                                mybir.AluOpType.max)
                        mx = M1
                    else:
                        vector.tensor_tensor(
                            map_(M2, gl, n, w),
                            map_(M1, gl, n, w),
                            vap(cc, s0, gl - 1, n, w=w),
                            mybir.AluOpType.max)
                        mx = M2
                    if fake_flat_d:
                        dg = dap(su, 0, 1, w=n * NWC)
                    else:
                        dg = dap(su, gl * DDIAG + d * NWC, n, DDIAG)
                    if d == ND - 1:
                        # output row: OB = D - M = acc[9, 19]
                        if u >= 2:
                            dneed = 16 * u
                            if dneed > dma_waited:
                                vector.wait_ge(dma_sem, dneed)
                                dma_waited = dneed
                        vector.tensor_tensor(
                            OB[su].ap(), dg, map_(mx, gl, n),
                            mybir.AluOpType.subtract).then_inc(dve_sem, 1)
                    else:
                        w = 4 if small_c else NWC
                        if small_c:
                            dg = dap(su, gl * DDIAG + d * NWC, n,
                                     DDIAG, w=4)
                        vector.tensor_tensor(
                            vap(cc, s0, gl, n, w=w),
                            map_(mx, gl, n, w),
                            dg, mybir.AluOpType.subtract)
    return nc


_NC_CACHE = None


def make_in_map(x: np.ndarray, negk: np.ndarray, b: int) -> dict:
    return {"x": x[b], "negk": negk}


def kernel(x: np.ndarray, kernels: np.ndarray) -> np.ndarray:
    global _NC_CACHE
    if _NC_CACHE is None:
        _NC_CACHE = _build_nc()
    nc = _NC_CACHE
    x = np.ascontiguousarray(x, dtype=np.float32)
    negk = np.ascontiguousarray(-np.asarray(kernels, dtype=np.float32))
    in_maps = [make_in_map(x, negk, b) for b in range(B)]
    res = run_bass_kernel_spmd(nc, in_maps, core_ids=list(range(B)))
    out = np.stack([res.results[b]["out"] for b in range(B)], axis=0)
    return out[:, :, :NW]
